# revision 3
# baseline (speedup 1.0000x reference)
"""TRN2 Bass kernel for nn_MAD_4612794876395 (retrieval_knn).

Math: with dist = softmax_k(-||pos_d - pos_r||) and sum_k dist = 1, the
reference output collapses to
    out[b,c] = wmem@adapt_w + adapt_b + wdiff@field_b.reshape(H,C)
             + sum_h wdiff[b,h] * (date@field_w)[b, h*C+c]
where wdiff[b,h] = sum_k dist[b,k]*diff[b,k,h].  The dominant term is the
137 GFLOP date@field_w product, computed on 8 NeuronCores tensor-parallel
over field_w's 65536 columns (64 h-values per core) as fp16 matmuls at
1 row/cycle.  The h-contraction (one multiply-add per matmul output
element) is split per 128-column h-block across three engines: DVE does
fused scalar_tensor_tensor from PSUM for blocks 0-1, Activation scales
blocks 2-3 out of PSUM into SBUF, and GPSIMD (no PSUM port) accumulates
those into a second SBUF accumulator.  Small terms are host numpy.
"""
import sys

sys.path.insert(0, "/opt/trn_rl_repo")

import numpy as np

N_DATA, F, H, C, K, B = 100000, 512, 512, 128, 8, 2048
NCORES = 8
HSH = H // NCORES          # 64 h-values per core
SH = HSH * C               # 8192 field_w cols per core
P = 128
NB = B // P                # 16 b-tiles
NS = SH // 512             # 16 col-slices of 512 (4 h-blocks each)

_NC = None
_LAST_IN_MAPS = None


def _build():
    import concourse.mybir as mybir
    import concourse.tile as tile
    from concourse import bacc

    nc = bacc.Bacc(None, target_bir_lowering=False, debug=False)
    dateT = nc.dram_tensor("dateT", [F, B], mybir.dt.float16, kind="ExternalInput")
    wdiff = nc.dram_tensor("wdiff", [B, HSH], mybir.dt.float32, kind="ExternalInput")
    fw = nc.dram_tensor("fw", [NS * 4 * P, 512], mybir.dt.float16,
                        kind="ExternalInput")
    partial_v = nc.dram_tensor("partial_v", [B, C], mybir.dt.float32,
                               kind="ExternalOutput")
    partial_p = nc.dram_tensor("partial_p", [B, C], mybir.dt.float32,
                               kind="ExternalOutput")

    with tile.TileContext(nc) as tc:
        with (
            tc.tile_pool(name="const", bufs=1) as cp,
            tc.tile_pool(name="fwp", bufs=4) as fwp,
            tc.tile_pool(name="tmp", bufs=8) as tp,
            tc.tile_pool(name="ps", bufs=8, space="PSUM") as ps,
        ):
            # resident fp16 dateT: 4 f-chunks of [128, B]
            dr = []
            for fc in range(4):
                d_t = cp.tile([P, B], mybir.dt.float16, name=f"d{fc}")
                nc.sync.dma_start(d_t[:], dateT[fc * P:(fc + 1) * P, :])
                dr.append(d_t)
            # per-b-tile wdiff columns + two per-engine accumulators
            wd, av, ag = [], [], []
            for t in range(NB):
                w_t = cp.tile([P, HSH], mybir.dt.float32, name=f"wd{t}")
                nc.sync.dma_start(w_t[:], wdiff[t * P:(t + 1) * P, :])
                wd.append(w_t)
                a1 = cp.tile([P, C], mybir.dt.float32, name=f"av{t}")
                nc.vector.memset(a1[:], 0.0)
                av.append(a1)
                a2 = cp.tile([P, C], mybir.dt.float32, name=f"ag{t}")
                nc.gpsimd.memset(a2[:], 0.0)
                ag.append(a2)

            for n in range(NS):
                fwr = []
                for fc in range(4):
                    f_t = fwp.tile([P, 512], mybir.dt.float16, name="f_t",
                                   tag=f"f{fc}")
                    nc.sync.dma_start(
                        f_t[:], fw[(n * 4 + fc) * P:(n * 4 + fc + 1) * P, :])
                    fwr.append(f_t)
                for t in range(NB):
                    g = ps.tile([P, 512], mybir.dt.float32, name="g", tag="g")
                    for fc in range(4):
                        nc.tensor.matmul(g[:], dr[fc][:, t * P:(t + 1) * P],
                                         fwr[fc][:], start=(fc == 0),
                                         stop=(fc == 3))
                    # h-blocks 0,1: fused multiply-add on DVE from PSUM
                    for l in range(2):
                        hcol = 4 * n + l
                        nc.vector.scalar_tensor_tensor(
                            out=av[t][:],
                            in0=g[:, l * C:(l + 1) * C],
                            scalar=wd[t][:, hcol:hcol + 1],
                            in1=av[t][:],
                            op0=mybir.AluOpType.mult,
                            op1=mybir.AluOpType.add,
                        )
                    # h-blocks 2,3: Act scales PSUM->SBUF; DVE (l=2) and
                    # GPSIMD (l=3, SBUF-only, TensorTensor is its supported
                    # op) accumulate into their own accumulators
                    for l in range(2, 4):
                        hcol = 4 * n + l
                        tm = tp.tile([P, C], mybir.dt.float32, name="tm",
                                     tag=f"tm{l}")
                        nc.scalar.activation(
                            tm[:], g[:, l * C:(l + 1) * C],
                            mybir.ActivationFunctionType.Copy,
                            bias=0.0, scale=wd[t][:, hcol:hcol + 1])
                        if l == 2:
                            nc.vector.tensor_add(av[t][:], tm[:], av[t][:])
                        else:
                            nc.gpsimd.tensor_add(ag[t][:], tm[:], ag[t][:])
            for t in range(NB):
                nc.sync.dma_start(partial_v[t * P:(t + 1) * P, :], av[t][:])
                nc.sync.dma_start(partial_p[t * P:(t + 1) * P, :], ag[t][:])
    nc.finalize()
    return nc


def kernel(idx, date, train_dates, mem, train_nns, pos_w, pos_b, field_w,
           field_b, adapt_w, adapt_b):
    global _NC, _LAST_IN_MAPS
    from concourse.bass_utils import run_bass_kernel_spmd

    idx = np.asarray(idx)
    date = np.asarray(date, dtype=np.float32)
    train_dates = np.asarray(train_dates, dtype=np.float32)
    mem = np.asarray(mem, dtype=np.float32)
    train_nns = np.asarray(train_nns)
    pos_w = np.asarray(pos_w, dtype=np.float32)
    pos_b = np.asarray(pos_b, dtype=np.float32)
    field_w = np.asarray(field_w, dtype=np.float32)
    field_b = np.asarray(field_b, dtype=np.float32)
    adapt_w = np.asarray(adapt_w, dtype=np.float32)
    adapt_b = np.asarray(adapt_b, dtype=np.float32)

    # ---- host phase 1 (small): dist, wdiff, const terms ----
    refs = train_nns[idx]                                   # [B, K]
    pos_d = date @ pos_w + pos_b                            # [B, H]
    pos_r = (train_dates[refs.reshape(-1)] @ pos_w + pos_b).reshape(B, K, H)
    diff = pos_d[:, None, :] - pos_r                        # [B, K, H]
    norm = np.sqrt((diff * diff).sum(-1))                   # [B, K]
    m = norm.min(axis=1, keepdims=True)
    e = np.exp(m - norm)
    dist = e / e.sum(axis=1, keepdims=True)                 # [B, K]
    wdiff = np.einsum("bk,bkh->bh", dist, diff).astype(np.float32)
    wmem = np.einsum("bk,bkc->bc", dist, mem[refs]).astype(np.float32)
    const = wmem @ adapt_w + adapt_b + wdiff @ field_b.reshape(H, C)

    # ---- device phase 2: grad-term, TP over the 65536 dim ----
    if _NC is None:
        _NC = _build()
    dateT16 = np.ascontiguousarray(date.T.astype(np.float16))
    in_maps = []
    for i in range(NCORES):
        fws = field_w[:, i * SH:(i + 1) * SH].astype(np.float16)  # [F, SH]
        # tile layout: row block (n*4+fc)*128 holds fw[fc*128:(fc+1)*128,
        # n*512:(n+1)*512] so every SBUF tile DMA is one contiguous read
        fwt = np.ascontiguousarray(
            fws.reshape(4, P, NS, 512).transpose(2, 0, 1, 3).reshape(-1, 512))
        in_maps.append({
            "dateT": dateT16,
            "wdiff": np.ascontiguousarray(wdiff[:, i * HSH:(i + 1) * HSH]),
            "fw": fwt,
        })
    _LAST_IN_MAPS = in_maps
    res = run_bass_kernel_spmd(_NC, in_maps, core_ids=list(range(NCORES)))
    grad_term = np.zeros((B, C), dtype=np.float32)
    for i in range(NCORES):
        grad_term += res.results[i]["partial_v"]
        grad_term += res.results[i]["partial_p"]
    return (const + grad_term).astype(np.float32)


def run_device(trace=False):
    """Re-run the device phase on the last inputs (test.py profiling)."""
    from concourse.bass_utils import run_bass_kernel_spmd
    assert _NC is not None and _LAST_IN_MAPS is not None
    return run_bass_kernel_spmd(_NC, _LAST_IN_MAPS,
                                core_ids=list(range(NCORES)), trace=trace)
